# revision 1
# baseline (speedup 1.0000x reference)
"""HardMoE classifier forward on 8 Trainium2 NeuronCores (Bass/Tile).

Math (per row b of cls_token [B, D]):
    logits[j]  = cls_token[b] . Wcat[j],  j in 0..17
                 (Wcat = concat(gate_w [6,D], expert_w.reshape(12, D)))
    choice     = argmax(logits[0:6] + gate_b)      (first-index tiebreak)
    out[b, l]  = logits[6 + 2*choice + l] + expert_b[choice, l]

Strategy: pure data parallel over batch (8 cores x 16384 rows), everything
in exact fp32 (fp32r measured at 1.5e-4 rel err = tf32-class; it would
flip ~50 argmax rows, so it is not used anywhere).

Per core, in super-iterations of 2048 rows (4 blocks x 512 rows):
  1. DMA 16 natural [128, 1024] row-tiles.
  2. PE-transpose each tile's 8 [128,128] chunks (fp32 transpose is
     bit-exact, ~105 ns/chunk); ACT/DVE copy psum->SBUF into xT layout.
  3. Column-tiled fp32 matmuls: the 128x128 PE array is split into 4
     column groups (tile_position=(0,32j)); group j accumulates
     logitsT [18, 512] for its own 512-row block with Wcat^T chunks
     [128,18] stationary and xT chunks [128,512] moving. The 4 groups
     stream concurrently: measured 231 ns per [128x18x512] fp32 matmul
     vs 860 ns untiled (3.7x).
  4. PE-transpose logitsT strips back to [rows, 18], then the vector
     engine does bias add, argmax with first-index tiebreak via a
     descending score, one-hot, and the 2-logit gather; DMA out.
"""

import json

import numpy as np

import concourse.bass as bass
import concourse.mybir as mybir
from concourse.bass_utils import run_bass_kernel_spmd
from concourse.tile import TileContext

F32 = mybir.dt.float32
ALU = mybir.AluOpType
AX = mybir.AxisListType

B, D, E, L = 131072, 1024, 6, 2
NCORES = 8
BLOC = B // NCORES            # 16384 rows per core
NJ = E + E * L                # 18 logit columns (6 gate + 12 expert)
KC = D // 128                 # 8 contraction chunks
NBLK = 4                      # PE column groups = 512-row blocks in flight
SUP = NBLK * 512              # 2048 rows per super-iteration
NSUP = BLOC // SUP            # 8 super-iterations per core

# ---------------------------------------------------------------------------
# Workaround: this walrus build supports only ONE sync wait per instruction,
# but Tile emits instructions (and its tail drain) with several. Split the
# extra monotonic (sem-ge) waits onto single-wait NoOps placed immediately
# before the instruction on the same engine.
# ---------------------------------------------------------------------------
_wsplit_counter = [0]


def _split_multiwaits(mod: dict) -> dict:
    for fn in mod.get("functions", []):
        for blk in fn.get("blocks", []):
            out = []
            changed = False
            for ins in blk.get("instructions", []):
                si = ins.get("sync_info") or {}
                waits = si.get("on_wait") or []
                if len(waits) > 1:
                    changed = True
                    ge = [w for w in waits if w.get("wait_mode", "").startswith("sem-ge")]
                    rest = [w for w in waits if not w.get("wait_mode", "").startswith("sem-ge")]
                    assert len(rest) <= 1, (
                        f"multiple non-monotonic waits on {ins.get('name')}: {rest}"
                    )
                    keep = rest[0] if rest else ge.pop()
                    for w in ge:
                        _wsplit_counter[0] += 1
                        out.append({
                            "debug": ins.get("debug", 0),
                            "engine": ins["engine"],
                            "ins": [],
                            "name": f"WSPLIT-{_wsplit_counter[0]}",
                            "opcode": "NoOp",
                            "outs": [],
                            "sync_info": {"on_update": [], "on_wait": [w]},
                        })
                    si["on_wait"] = [keep]
                    ins["sync_info"] = si
                out.append(ins)
            if changed:
                blk["instructions"] = out
    return mod


_orig_to_json_bytes = bass.Bass.to_json_bytes


def _patched_to_json_bytes(self) -> bytes:
    mod = json.loads(_orig_to_json_bytes(self))
    return json.dumps(_split_multiwaits(mod)).encode()


if bass.Bass.to_json_bytes is not _patched_to_json_bytes:
    bass.Bass.to_json_bytes = _patched_to_json_bytes


# ---------------------------------------------------------------------------
# Device kernel (one NeuronCore's shard)
# ---------------------------------------------------------------------------

def _build_nc(time_loop: int = 0) -> bass.Bass:
    nc = bass.Bass(name="hardmoe")
    x = nc.dram_tensor("x", [BLOC, D], F32, kind="ExternalInput")
    wt = nc.dram_tensor("wt", [KC, 128, NJ], F32, kind="ExternalInput")
    bias = nc.dram_tensor("bias", [128, NJ], F32, kind="ExternalInput")
    desc = nc.dram_tensor("desc", [128, E], F32, kind="ExternalInput")
    idt = nc.dram_tensor("idt", [128, 128], F32, kind="ExternalInput")
    idt32 = nc.dram_tensor("idt32", [128, NJ], F32, kind="ExternalInput")
    out = nc.dram_tensor("out", [BLOC, L], F32, kind="ExternalOutput")

    xv = x.rearrange("(n p) d -> n p d", p=128)          # [128 tiles, 128, 1024]

    with TileContext(nc) as tc:
        with tc.tile_pool(name="const", bufs=1) as cpool, \
             tc.tile_pool(name="xin", bufs=8) as xpool, \
             tc.tile_pool(name="xt", bufs=2) as xtpool, \
             tc.tile_pool(name="pstr", bufs=2, space="PSUM") as pstr_pool, \
             tc.tile_pool(name="psmm", bufs=1, space="PSUM") as psmm_pool, \
             tc.tile_pool(name="pstb", bufs=2, space="PSUM") as pstb_pool, \
             tc.tile_pool(name="lsb", bufs=2) as lpool, \
             tc.tile_pool(name="sel", bufs=2) as selpool:

            wt_sb = cpool.tile([128, KC, NJ], F32)
            nc.sync.dma_start(wt_sb[:], wt.rearrange("k p j -> p k j"))
            bias_sb = cpool.tile([128, NJ], F32)
            nc.sync.dma_start(bias_sb[:], bias[:])
            desc_sb = cpool.tile([128, E], F32)
            nc.sync.dma_start(desc_sb[:], desc[:])
            ident = cpool.tile([128, 128], F32)
            nc.sync.dma_start(ident[:], idt[:])
            ident32 = cpool.tile([128, NJ], F32)
            nc.sync.dma_start(ident32[:], idt32[:])

            def body():
                # xts[parity] = [128, KC, SUP] transposed super-tile buffer
                def stage_super(s: int):
                    """DMA + transpose the 16 tiles of super s into xts buf."""
                    xts = xtpool.tile([128, KC, SUP], F32, tag="xts")
                    for tt in range(SUP // 128):          # 16 row-tiles
                        t = s * (SUP // 128) + tt
                        xb = xpool.tile([128, D], F32, tag="xb")
                        nc.sync.dma_start(xb[:], xv[t])
                        for h in range(2):
                            pst = pstr_pool.tile([128, 512], F32, tag="pst")
                            for q in range(4):
                                k = h * 4 + q
                                nc.tensor.transpose(
                                    pst[:, q * 128:(q + 1) * 128],
                                    xb[:, k * 128:(k + 1) * 128],
                                    ident[:],
                                )
                            dst = xts[:, h * 4:(h + 1) * 4, tt * 128:(tt + 1) * 128]
                            if (tt + h) % 2 == 0:
                                nc.scalar.copy(dst, pst[:])
                            else:
                                nc.vector.tensor_copy(dst, pst[:])
                    return xts

                live = {0: stage_super(0)}

                for s in range(NSUP):
                    if s + 1 < NSUP:
                        live[s + 1] = stage_super(s + 1)
                    xts = live.pop(s)

                    # column-tiled matmuls: group j <-> rows [32j, 32j+18) of
                    # psum, block j <-> xts columns [512j, 512(j+1))
                    # start=True clears has_written for the whole bank, so each
                    # column group accumulates in its own PSUM bank.
                    ps_mm = [
                        psmm_pool.tile([128, 512], F32, tag=f"ps_mm{j}",
                                       name=f"ps_mm{j}")
                        for j in range(NBLK)
                    ]
                    for k in range(KC):
                        for j in range(NBLK):
                            nc.tensor.matmul(
                                ps_mm[j][32 * j:32 * j + NJ, :],
                                wt_sb[:, k],
                                xts[:, k, 512 * j:512 * (j + 1)],
                                start=(k == 0),
                                stop=(k == KC - 1),
                                tile_position=(0, 32 * j),
                            )
                    l_sb = lpool.tile([128, 512], F32, tag="l_sb")
                    for j in range(NBLK):
                        nc.scalar.copy(
                            l_sb[32 * j:32 * j + NJ, :],
                            ps_mm[j][32 * j:32 * j + NJ, :],
                        )

                    # transpose logitsT strips back to [rows, 18] and select,
                    # one megagroup = 2 blocks = 1024 rows
                    for mg in range(2):
                        tp = pstb_pool.tile([128, 8, NJ], F32, tag="tp")
                        for half in range(8):             # 8 x 128-row slices
                            j = mg * 2 + half // 4
                            c = half % 4
                            nc.tensor.matmul(
                                tp[:, half, :],
                                l_sb[32 * j:32 * j + NJ, c * 128:(c + 1) * 128],
                                ident32[32 * j:32 * j + NJ, :],
                                is_transpose=True,
                                tile_position=(32 * j, 0),
                            )
                        A = selpool.tile([128, 8, NJ], F32, tag="A")
                        nc.scalar.copy(A[:], tp[:])
                        nc.vector.tensor_tensor(
                            A[:], A[:],
                            bias_sb[:, None, :].to_broadcast([128, 8, NJ]),
                            ALU.add,
                        )
                        gate = A[:, :, 0:E]
                        m = selpool.tile([128, 8], F32, tag="m")
                        nc.vector.tensor_reduce(m[:], gate, AX.X, ALU.max)
                        eq = selpool.tile([128, 8, E], F32, tag="eq")
                        nc.vector.tensor_tensor(
                            eq[:], gate, m[:, :, None].to_broadcast([128, 8, E]),
                            ALU.is_ge,
                        )
                        nc.vector.tensor_tensor(
                            eq[:], eq[:],
                            desc_sb[:, None, :].to_broadcast([128, 8, E]),
                            ALU.mult,
                        )
                        nc.vector.tensor_reduce(m[:], eq[:], AX.X, ALU.max)
                        onehot = selpool.tile([128, 8, E], F32, tag="onehot")
                        nc.vector.tensor_tensor(
                            onehot[:], eq[:], m[:, :, None].to_broadcast([128, 8, E]),
                            ALU.is_equal,
                        )
                        outs = selpool.tile([128, 8, L], F32, tag="outs")
                        sel = selpool.tile([128, 8, E], F32, tag="sel")
                        for l in range(L):
                            nc.vector.tensor_tensor(
                                sel[:], onehot[:], A[:, :, E + l::L], ALU.mult
                            )
                            nc.vector.tensor_reduce(
                                outs[:, :, l], sel[:], AX.X, ALU.add
                            )
                        r0 = (s * NBLK + mg * 2) * 512
                        nc.sync.dma_start(
                            out[r0:r0 + 1024, :].rearrange("(g p) l -> p g l", p=128),
                            outs[:],
                        )

            if time_loop:
                with tc.For_i(0, time_loop, 1, name="timing") as _i:
                    body()
            else:
                body()
    return nc


_cached = None


def _get_nc() -> bass.Bass:
    global _cached
    if _cached is None:
        _cached = _build_nc()
    return _cached


# ---------------------------------------------------------------------------
# Host wrapper
# ---------------------------------------------------------------------------

def _host_inputs(cls_token, gate_w, gate_b, expert_w, expert_b):
    x = np.ascontiguousarray(np.asarray(cls_token, dtype=np.float32))
    gw = np.asarray(gate_w, dtype=np.float32)
    gb = np.asarray(gate_b, dtype=np.float32)
    ew = np.asarray(expert_w, dtype=np.float32)
    eb = np.asarray(expert_b, dtype=np.float32)
    assert x.shape == (B, D), x.shape

    wcat = np.concatenate([gw, ew.reshape(E * L, D)], axis=0)      # [18, D]
    wt_in = np.ascontiguousarray(wcat.T).reshape(KC, 128, NJ)
    bias_in = np.ascontiguousarray(np.broadcast_to(
        np.concatenate([gb, eb.reshape(E * L)])[None, :], (128, NJ)))
    desc_in = np.ascontiguousarray(np.broadcast_to(
        (E - np.arange(E, dtype=np.float32))[None, :], (128, E)))
    idt_in = np.eye(128, dtype=np.float32)
    idt32_in = np.zeros((128, NJ), np.float32)
    for p in range(128):
        if p % 32 < NJ:
            idt32_in[p, p % 32] = 1.0

    in_maps = []
    for c in range(NCORES):
        in_maps.append({
            "x": x[c * BLOC:(c + 1) * BLOC],
            "wt": wt_in,
            "bias": bias_in,
            "desc": desc_in,
            "idt": idt_in,
            "idt32": idt32_in,
        })
    return in_maps


def kernel(cls_token, gate_w, gate_b, expert_w, expert_b) -> np.ndarray:
    in_maps = _host_inputs(cls_token, gate_w, gate_b, expert_w, expert_b)
    res = run_bass_kernel_spmd(_get_nc(), in_maps, core_ids=list(range(NCORES)))
    return np.concatenate([r["out"] for r in res.results], axis=0)



# revision 2
# speedup vs baseline: 1.7997x; 1.7997x over previous
"""HardMoE classifier forward on 8 Trainium2 NeuronCores (Bass/Tile).

Math (per row b of cls_token [B, D]):
    logits[j]  = cls_token[b] . Wcat[j],  j in 0..17
                 (Wcat = concat(gate_w [6,D], expert_w.reshape(12, D)))
    choice     = argmax(logits[0:6] + gate_b)      (first-index tiebreak)
    out[b, l]  = logits[6 + 2*choice + l] + expert_b[choice, l]

Strategy: pure data parallel over batch (8 cores x 16384 rows), exact fp32.

DMA layout (the critical part on this platform): partition p owns the
contiguous row block [128p, 128(p+1)); each input DMA loads [128, 8, 1024]
(4 MiB, 32 KB contiguous per partition) so descriptors are few and large.
Outputs are staged in SBUF [128, 128, 2] and written by ONE contiguous DMA
per pass (row = 128p + m).  The previous revision used 128 small input DMAs
plus 16 scattered 8-byte-element output DMAs per core-pass; on 8 concurrent
cores that descriptor storm ran 4.6x slower end to end.

Compute per core, in super-iterations of 2048 rows (4 blocks x 512 rows):
  1. PE-transpose each [128,128] chunk (fp32 transpose is bit-exact);
     ACT/DVE copy psum->SBUF into xT layout.
  2. Column-tiled fp32 matmuls: PE split into 4 column groups
     (tile_position=(0,32j)); group j accumulates logitsT [18, 512] for its
     512-row block with Wcat^T chunks [128,18] stationary.
  3. PE-transpose logitsT strips back to [rows, 18]; vector engine does
     bias add, argmax with first-index tiebreak, one-hot, 2-logit gather.
"""

import json

import numpy as np

import concourse.bass as bass
import concourse.mybir as mybir
from concourse.bass_utils import run_bass_kernel_spmd
from concourse.tile import TileContext

F32 = mybir.dt.float32
ALU = mybir.AluOpType
AX = mybir.AxisListType

B, D, E, L = 131072, 1024, 6, 2
NCORES = 8
BLOC = B // NCORES            # 16384 rows per core
NJ = E + E * L                # 18 logit columns (6 gate + 12 expert)
KC = D // 128                 # 8 contraction chunks
NBLK = 4                      # PE column groups = 512-row blocks in flight
SUP = NBLK * 512              # 2048 rows per super-iteration
NSUP = BLOC // SUP            # 8 super-iterations per core
GT = 8                        # row-tiles per input DMA ([128, GT*1024] = 4 MiB)

# ---------------------------------------------------------------------------
# Workaround: this walrus build supports only ONE sync wait per instruction,
# but Tile emits instructions (and its tail drain) with several. Split the
# extra monotonic (sem-ge) waits onto single-wait NoOps placed immediately
# before the instruction on the same engine.
# ---------------------------------------------------------------------------
_wsplit_counter = [0]


def _split_multiwaits(mod: dict) -> dict:
    for fn in mod.get("functions", []):
        for blk in fn.get("blocks", []):
            out = []
            changed = False
            for ins in blk.get("instructions", []):
                si = ins.get("sync_info") or {}
                waits = si.get("on_wait") or []
                if len(waits) > 1:
                    changed = True
                    ge = [w for w in waits if w.get("wait_mode", "").startswith("sem-ge")]
                    rest = [w for w in waits if not w.get("wait_mode", "").startswith("sem-ge")]
                    assert len(rest) <= 1, (
                        f"multiple non-monotonic waits on {ins.get('name')}: {rest}"
                    )
                    keep = rest[0] if rest else ge.pop()
                    for w in ge:
                        _wsplit_counter[0] += 1
                        out.append({
                            "debug": ins.get("debug", 0),
                            "engine": ins["engine"],
                            "ins": [],
                            "name": f"WSPLIT-{_wsplit_counter[0]}",
                            "opcode": "NoOp",
                            "outs": [],
                            "sync_info": {"on_update": [], "on_wait": [w]},
                        })
                    si["on_wait"] = [keep]
                    ins["sync_info"] = si
                out.append(ins)
            if changed:
                blk["instructions"] = out
    return mod


_orig_to_json_bytes = bass.Bass.to_json_bytes


def _patched_to_json_bytes(self) -> bytes:
    mod = json.loads(_orig_to_json_bytes(self))
    return json.dumps(_split_multiwaits(mod)).encode()


if bass.Bass.to_json_bytes is not _patched_to_json_bytes:
    bass.Bass.to_json_bytes = _patched_to_json_bytes


# ---------------------------------------------------------------------------
# Device kernel (one NeuronCore's shard)
# ---------------------------------------------------------------------------

def _build_nc(time_loop: int = 0) -> bass.Bass:
    nc = bass.Bass(name="hardmoe")
    x = nc.dram_tensor("x", [BLOC, D], F32, kind="ExternalInput")
    wt = nc.dram_tensor("wt", [KC, 128, NJ], F32, kind="ExternalInput")
    bias = nc.dram_tensor("bias", [128, NJ], F32, kind="ExternalInput")
    desc = nc.dram_tensor("desc", [128, E], F32, kind="ExternalInput")
    idt = nc.dram_tensor("idt", [128, 128], F32, kind="ExternalInput")
    idt32 = nc.dram_tensor("idt32", [128, NJ], F32, kind="ExternalInput")
    out = nc.dram_tensor("out", [BLOC, L], F32, kind="ExternalOutput")

    # contiguous view: partition p owns rows [128p, 128(p+1)); DMA n loads
    # rows r = 128p + GT*n + g as [128, GT, D], 32 KB contiguous/partition
    xv = x.rearrange("(p n g) d -> n p g d", p=128, g=GT)

    with TileContext(nc) as tc:
        with tc.tile_pool(name="const", bufs=1) as cpool, \
             tc.tile_pool(name="xin", bufs=2) as xpool, \
             tc.tile_pool(name="xt", bufs=2) as xtpool, \
             tc.tile_pool(name="pstr", bufs=2, space="PSUM") as pstr_pool, \
             tc.tile_pool(name="psmm", bufs=1, space="PSUM") as psmm_pool, \
             tc.tile_pool(name="pstb", bufs=2, space="PSUM") as pstb_pool, \
             tc.tile_pool(name="lsb", bufs=2) as lpool, \
             tc.tile_pool(name="sel", bufs=2) as selpool, \
             tc.tile_pool(name="ost", bufs=2) as opool:

            wt_sb = cpool.tile([128, KC, NJ], F32)
            nc.sync.dma_start(wt_sb[:], wt.rearrange("k p j -> p k j"))
            bias_sb = cpool.tile([128, NJ], F32)
            nc.sync.dma_start(bias_sb[:], bias[:])
            desc_sb = cpool.tile([128, E], F32)
            nc.sync.dma_start(desc_sb[:], desc[:])
            ident = cpool.tile([128, 128], F32)
            nc.sync.dma_start(ident[:], idt[:])
            ident32 = cpool.tile([128, NJ], F32)
            nc.sync.dma_start(ident32[:], idt32[:])

            def body():
                ost = opool.tile([128, BLOC // 128, L], F32, tag="ost")

                def stage_super(s: int):
                    """One super = 2048 rows = 2 big DMAs of GT=8 tiles."""
                    xts = xtpool.tile([128, KC, SUP], F32, tag="xts")
                    for half in range(2):
                        xb = xpool.tile([128, GT, D], F32, tag="xb")
                        nc.sync.dma_start(xb[:], xv[s * 2 + half])
                        for g in range(GT):
                            tt = half * GT + g
                            for h in range(2):
                                pst = pstr_pool.tile([128, 512], F32, tag="pst")
                                for q in range(4):
                                    k = h * 4 + q
                                    nc.tensor.transpose(
                                        pst[:, q * 128:(q + 1) * 128],
                                        xb[:, g, k * 128:(k + 1) * 128],
                                        ident[:],
                                    )
                                dst = xts[:, h * 4:(h + 1) * 4,
                                          tt * 128:(tt + 1) * 128]
                                if (tt + h) % 2 == 0:
                                    nc.scalar.copy(dst, pst[:])
                                else:
                                    nc.vector.tensor_copy(dst, pst[:])
                    return xts

                live = {0: stage_super(0)}

                for s in range(NSUP):
                    if s + 1 < NSUP:
                        live[s + 1] = stage_super(s + 1)
                    xts = live.pop(s)

                    # column-tiled matmuls: group j <-> rows [32j, 32j+18) of
                    # psum, block j <-> xts columns [512j, 512(j+1))
                    ps_mm = [
                        psmm_pool.tile([128, 512], F32, tag=f"ps_mm{j}",
                                       name=f"ps_mm{j}")
                        for j in range(NBLK)
                    ]
                    for k in range(KC):
                        for j in range(NBLK):
                            nc.tensor.matmul(
                                ps_mm[j][32 * j:32 * j + NJ, :],
                                wt_sb[:, k],
                                xts[:, k, 512 * j:512 * (j + 1)],
                                start=(k == 0),
                                stop=(k == KC - 1),
                                tile_position=(0, 32 * j),
                            )
                    l_sb = lpool.tile([128, 512], F32, tag="l_sb")
                    for j in range(NBLK):
                        nc.scalar.copy(
                            l_sb[32 * j:32 * j + NJ, :],
                            ps_mm[j][32 * j:32 * j + NJ, :],
                        )

                    # transpose logitsT strips back to [rows, 18] and select,
                    # one megagroup = 2 blocks = 1024 rows
                    for mg in range(2):
                        tp = pstb_pool.tile([128, 8, NJ], F32, tag="tp")
                        for half in range(8):             # 8 x 128-row slices
                            j = mg * 2 + half // 4
                            c = half % 4
                            nc.tensor.matmul(
                                tp[:, half, :],
                                l_sb[32 * j:32 * j + NJ, c * 128:(c + 1) * 128],
                                ident32[32 * j:32 * j + NJ, :],
                                is_transpose=True,
                                tile_position=(32 * j, 0),
                            )
                        A = selpool.tile([128, 8, NJ], F32, tag="A")
                        nc.scalar.copy(A[:], tp[:])
                        nc.vector.tensor_tensor(
                            A[:], A[:],
                            bias_sb[:, None, :].to_broadcast([128, 8, NJ]),
                            ALU.add,
                        )
                        gate = A[:, :, 0:E]
                        m = selpool.tile([128, 8], F32, tag="m")
                        nc.vector.tensor_reduce(m[:], gate, AX.X, ALU.max)
                        eq = selpool.tile([128, 8, E], F32, tag="eq")
                        nc.vector.tensor_tensor(
                            eq[:], gate, m[:, :, None].to_broadcast([128, 8, E]),
                            ALU.is_ge,
                        )
                        nc.vector.tensor_tensor(
                            eq[:], eq[:],
                            desc_sb[:, None, :].to_broadcast([128, 8, E]),
                            ALU.mult,
                        )
                        nc.vector.tensor_reduce(m[:], eq[:], AX.X, ALU.max)
                        onehot = selpool.tile([128, 8, E], F32, tag="onehot")
                        nc.vector.tensor_tensor(
                            onehot[:], eq[:],
                            m[:, :, None].to_broadcast([128, 8, E]),
                            ALU.is_equal,
                        )
                        sel = selpool.tile([128, 8, E], F32, tag="sel")
                        # rows r = 128p + 16s + 8mg + half -> ost[p, m, l]
                        c0 = s * (SUP // 128) + mg * 8
                        for l in range(L):
                            nc.vector.tensor_tensor(
                                sel[:], onehot[:], A[:, :, E + l::L], ALU.mult
                            )
                            nc.vector.tensor_reduce(
                                ost[:, c0:c0 + 8, l], sel[:], AX.X, ALU.add
                            )
                # one contiguous output DMA per pass (row = 128p + m)
                nc.sync.dma_start(
                    out.rearrange("(p m) l -> p m l", p=128), ost[:])

            if time_loop:
                with tc.For_i(0, time_loop, 1, name="timing") as _i:
                    body()
            else:
                body()
    return nc


_cached = None


def _get_nc() -> bass.Bass:
    global _cached
    if _cached is None:
        _cached = _build_nc()
    return _cached


# ---------------------------------------------------------------------------
# Host wrapper
# ---------------------------------------------------------------------------

def _host_inputs(cls_token, gate_w, gate_b, expert_w, expert_b):
    x = np.ascontiguousarray(np.asarray(cls_token, dtype=np.float32))
    gw = np.asarray(gate_w, dtype=np.float32)
    gb = np.asarray(gate_b, dtype=np.float32)
    ew = np.asarray(expert_w, dtype=np.float32)
    eb = np.asarray(expert_b, dtype=np.float32)
    assert x.shape == (B, D), x.shape

    wcat = np.concatenate([gw, ew.reshape(E * L, D)], axis=0)      # [18, D]
    wt_in = np.ascontiguousarray(wcat.T).reshape(KC, 128, NJ)
    bias_in = np.ascontiguousarray(np.broadcast_to(
        np.concatenate([gb, eb.reshape(E * L)])[None, :], (128, NJ)))
    desc_in = np.ascontiguousarray(np.broadcast_to(
        (E - np.arange(E, dtype=np.float32))[None, :], (128, E)))
    idt_in = np.eye(128, dtype=np.float32)
    idt32_in = np.zeros((128, NJ), np.float32)
    for p in range(128):
        if p % 32 < NJ:
            idt32_in[p, p % 32] = 1.0

    in_maps = []
    for c in range(NCORES):
        in_maps.append({
            "x": x[c * BLOC:(c + 1) * BLOC],
            "wt": wt_in,
            "bias": bias_in,
            "desc": desc_in,
            "idt": idt_in,
            "idt32": idt32_in,
        })
    return in_maps


def kernel(cls_token, gate_w, gate_b, expert_w, expert_b) -> np.ndarray:
    in_maps = _host_inputs(cls_token, gate_w, gate_b, expert_w, expert_b)
    res = run_bass_kernel_spmd(_get_nc(), in_maps, core_ids=list(range(NCORES)))
    return np.concatenate([r["out"] for r in res.results], axis=0)
